# revision 11
# baseline (speedup 1.0000x reference)
"""OIM loss kernel for Trainium2, 8 NeuronCores, data-parallel over the roi dim.

Math (per reference):
    bank   = concat([lut, cq], 0)                      # [L=10532, D=256]
    logits = (inputs @ bank.T) * reliability * 30.0    # [N=8192, L]
    loss   = mean over rows with label != 5554 of
             logsumexp(logits[r]) - logits[r, label[r]]

Distribution: rows split 1024/core across 8 cores; the (reliability*30)-scaled
bank is replicated in fp8-e4m3 (inputs pre-scaled x16 so both fp8 operands sit
near unit variance; the exp folds 1/16 back in via its scale field).  Each
core returns [sum of masked nll, n_valid]; the host combines.

Per-core pipeline (PE streams fp8 DoubleRow matmuls into 2 rotating PSUM
col-blocks; the exp+row-sum over the 10.5K x 1K logits is split between ACT
and DVE so the two PSUM consumers overlap):
  A-blocks: ACT exp (scale fused) -> throwaway SBUF scratch, with the fused
            row-sum accumulator producing the block sum (sum is ~free).
  S-blocks: DVE Schraudolph bit-exp (affine f32->int16 write of the bf16 bit
            pattern) into a per-(cb-pair,rt) es row; after both halves land,
            ONE fused DVE reduce over the pair (DVE reduce is ~0.4ns/elem
            plus a large ~0.8us fixed cost, so wide fused sums win).
  ln(sumexp) via the reverse bit trick on DVE (no ACT Ln-table swap), picked
  logits as one fused dot, final cross-partition reduce on the PE.
"""

import numpy as np
import ml_dtypes

N = 8192
D = 256
L = 10532  # 5532 + 5000
NCORES = 8
NSH = N // NCORES     # 1024 rows per core
P = 128               # partitions
RT = NSH // P         # 8 row tiles per core
KC = D // P           # 2 contraction chunks (DoubleRow pair)
IGNORE = 5554
OIM_SCALAR = 30.0
FP8_SCALE = 16.0      # inputs pre-scaled by this; exp() divides it back out

# col-blocks; small first blocks start the exp pipeline while banks stream in
WIDTHS = [1024, 1024, 2048, 2048, 2048, 2048, 292]
SUM_GPS = set(range(10, 48, 5)) | {49, 51, 53, 55}  # row-sums on a gpsimd tree
SCHRAU_BLOCKS = {18, 30, 42}  # sparse DVE bit-exp blocks (sums on gpsimd)
OFFS = [sum(WIDTHS[:i]) for i in range(len(WIDTHS))]
NCB = len(WIDTHS)
assert sum(WIDTHS) == L

# ACT is the sole PSUM consumer (keeps the 2-slot PSUM rotation smooth and
# ACT-paced, like the proven baseline); row-sums go downstream: most blocks
# exp into an es slot summed by a DVE reduce, every 5th uses the fused ACT
# accumulator, and SUM_GPS blocks''' reduces move to a gpsimd add-tree to keep
# DVE well below the ACT pace.

# Schraudolph exp on bf16 bit patterns: i16 = trunc(a*raw + b) viewed as bf16
# approximates exp(raw/16).  b tuned so block sums are unbiased under the
# truncating float->int convert (numpy/CoreSim; hw round-to-nearest would
# shift exp by +0.27%, still far inside tolerance).
SCHRAU_A = 128.0 / np.log(2.0) / FP8_SCALE
SCHRAU_B = 16249.136
# ln(x) ~= float_bits_as_int(x) * ln2/2^23 - C  (same trick in reverse)
FLN_K = float(np.log(2.0) / 2**23)
FLN_C = 88.02637566918142

BF16 = ml_dtypes.bfloat16
FP8 = ml_dtypes.float8_e4m3

_CACHE = {}


def _build(debug=False):
    import concourse.bacc as bacc
    import concourse.tile as tile
    from concourse import mybir

    fp8 = mybir.dt.float8e4
    bf16 = mybir.dt.bfloat16
    f32 = mybir.dt.float32
    i16 = mybir.dt.int16
    i32 = mybir.dt.int32
    AF = mybir.ActivationFunctionType
    ALU = mybir.AluOpType
    AX = mybir.AxisListType
    DR = mybir.MatmulPerfMode.DoubleRow

    nc = bacc.Bacc(
        "TRN2", target_bir_lowering=False, debug=debug, enable_partition_id=False
    )

    # element (p, rt, k, c) = x16[rt*128 + c, k*128 + p]
    d_inp = nc.dram_tensor("inp", [P, RT, KC, P], fp8, kind="ExternalInput").ap()
    # element (p, k, j) = scaled[j, k*128 + p]
    d_bank = nc.dram_tensor("bank", [P, KC, L], fp8, kind="ExternalInput").ap()
    d_rows = nc.dram_tensor("rows", [P, RT, D], bf16, kind="ExternalInput").ap()
    d_bsel = nc.dram_tensor("bsel", [P, RT, D], bf16, kind="ExternalInput").ap()
    d_mask = nc.dram_tensor("mask", [P, RT], f32, kind="ExternalInput").ap()
    d_out = nc.dram_tensor("out", [1, 2], f32, kind="ExternalOutput").ap()

    with tile.TileContext(nc) as tc:
        with (
            tc.tile_pool(name="const", bufs=1) as const,
            tc.tile_pool(name="work", bufs=2) as work,
            tc.tile_pool(name="psum", bufs=2, space="PSUM") as psum,
        ):
            # --- resident inputs ---
            inp_sb = const.tile([P, RT, KC, P], fp8)
            bank_sb = [
                const.tile([P, KC, WIDTHS[cb]], fp8, tag=f"bk{cb}", name=f"bk{cb}")
                for cb in range(NCB)
            ]
            rows_sb = const.tile([P, RT, D], bf16)
            bsel_sb = const.tile([P, RT, D], bf16)
            mask_sb = const.tile([P, RT], f32)

            # --- startup DMA: critical pieces fan out on 3 engine queues ---
            nc.sync.dma_start(out=inp_sb[:, 0], in_=d_inp[:, 0])
            nc.sync.dma_start(out=bank_sb[0][:, :, 0:512], in_=d_bank[:, :, 0:512])
            nc.scalar.dma_start(
                out=bank_sb[0][:, :, 512:1024], in_=d_bank[:, :, 512:1024]
            )
            nc.scalar.dma_start(
                out=bank_sb[1], in_=d_bank[:, :, OFFS[1] : OFFS[1] + WIDTHS[1]]
            )
            nc.gpsimd.dma_start(out=inp_sb[:, 1:], in_=d_inp[:, 1:])
            nc.gpsimd.dma_start(
                out=bank_sb[2], in_=d_bank[:, :, OFFS[2] : OFFS[2] + WIDTHS[2]]
            )
            late_dmas = []  # (anchor block idx, inst)
            for cb in range(3, NCB):
                anchor = (cb - 3) * 8 + 6
                late_dmas.append(
                    (
                        anchor,
                        nc.gpsimd.dma_start(
                            out=bank_sb[cb],
                            in_=d_bank[:, :, OFFS[cb] : OFFS[cb] + WIDTHS[cb]],
                        ),
                    )
                )
            late_dmas.append((8, nc.sync.dma_start(out=rows_sb, in_=d_rows)))
            late_dmas.append((10, nc.sync.dma_start(out=bsel_sb, in_=d_bsel)))
            late_dmas.append((12, nc.sync.dma_start(out=mask_sb, in_=d_mask)))

            # --- ACT exp-table preload: tiny dummy exp scheduled first ---
            tiny = const.tile([P, 1], f32)
            nc.vector.memset(tiny, 0.0)
            tiny_o = const.tile([P, 1], f32)
            nc.scalar.activation(out=tiny_o, in_=tiny, func=AF.Exp)

            # --- PE warmup: ramp the HAM clock gate during the DMA wait ---
            wsrc = const.tile([P, KC, 512], fp8)
            nc.vector.memset(wsrc, 0.25)
            pw = psum.tile([P, 2048], f32, tag="ps", name="warm")
            warm_mms = []
            for i in range(3):
                m = nc.tensor.matmul(
                    pw[:, 0:512],
                    wsrc[:, :, 0:P],
                    wsrc,
                    start=True,
                    stop=True,
                    perf_mode=DR,
                )
                if warm_mms:
                    tile.add_dep_helper(m.ins, warm_mms[-1].ins, reason="warm order")
                warm_mms.append(m)

            # --- picked logit: one fused dot over all row tiles on DVE ---
            picked = const.tile([P, RT], f32)
            dots = const.tile([P, RT, D], bf16)
            nc.vector.tensor_mul(dots, rows_sb, bsel_sb)
            nc.vector.tensor_reduce(out=picked, in_=dots, axis=AX.X, op=ALU.add)

            def gps_tree(esl, w, acc):
                """Row-sum on gpsimd: in-place add tree w -> 16, DVE finishes."""
                hw_ = w // 2
                while hw_ >= 16:
                    nc.gpsimd.tensor_tensor(
                        esl[:, :hw_], esl[:, :hw_], esl[:, hw_ : 2 * hw_], op=ALU.add
                    )
                    hw_ //= 2
                nc.vector.tensor_reduce(
                    out=acc, in_=esl[:, :16], axis=AX.X, op=ALU.add
                )

            # --- main loop ---
            blocksums = const.tile([P, RT * NCB], f32)
            es = work.tile([P, 8, 2048], bf16, bufs=1)
            # A-path exp target, never read; 2 rotating slots so the psum
            # WAR releases at ACTIVATE completion (not after the accumulator
            # read) and consecutive A-exps don't chain on a WAW drain
            trash = const.tile([P, 2, 2048], bf16)
            nes = 0
            exps = []  # per-block psum-consumer instr, for DMA anchors
            for cb in range(NCB):
                w = WIDTHS[cb]
                nb = (w + 511) // 512
                for rt in range(RT):
                    ps = psum.tile([P, 2048], f32, tag="ps", name=f"ps_{cb}_{rt}")
                    lhsT = inp_sb[:, rt]
                    for b in range(nb):
                        bw = min(512, w - b * 512)
                        nc.tensor.matmul(
                            ps[:, b * 512 : b * 512 + bw],
                            lhsT,
                            bank_sb[cb][:, :, b * 512 : b * 512 + bw],
                            start=True,
                            stop=True,
                            perf_mode=DR,
                        )
                    acc = blocksums[:, rt * NCB + cb : rt * NCB + cb + 1]
                    bidx = len(exps)
                    if bidx in SCHRAU_BLOCKS:
                        esl = es[:, nes % 8]
                        nes += 1
                        a = nc.vector.tensor_scalar(
                            out=esl[:, :w].bitcast(i16),
                            in0=ps[:, :w],
                            scalar1=SCHRAU_A,
                            scalar2=SCHRAU_B,
                            op0=ALU.mult,
                            op1=ALU.add,
                        )
                        gps_tree(esl, w, acc)
                    elif bidx % 5 == 4 and bidx not in SUM_GPS:
                        a = nc.scalar.activation(
                            out=trash[:, bidx % 2, :w],
                            in_=ps[:, :w],
                            func=AF.Exp,
                            scale=1.0 / FP8_SCALE,
                            accum_out=acc,
                        )
                    else:
                        esl = es[:, nes % 8]
                        nes += 1
                        a = nc.scalar.activation(
                            out=esl[:, :w],
                            in_=ps[:, :w],
                            func=AF.Exp,
                            scale=1.0 / FP8_SCALE,
                        )
                        if bidx in SUM_GPS:
                            gps_tree(esl, w, acc)
                        else:
                            nc.vector.tensor_reduce(
                                out=acc, in_=esl[:, :w], axis=AX.X, op=ALU.add
                            )
                    exps.append(a)
            for anchor, dma in late_dmas:
                tile.add_dep_helper(
                    dma.ins,
                    exps[anchor].ins,
                    reason="hold non-critical DMAs off the startup window",
                )

            # --- tail: nll = ln(sumexp) - picked, masked sums (no ACT) ---
            sumexp = const.tile([P, RT], f32)
            nc.vector.tensor_reduce(
                out=sumexp,
                in_=blocksums.rearrange("p (r c) -> p r c", c=NCB),
                axis=AX.X,
                op=ALU.add,
            )
            lnse = const.tile([P, RT], f32)
            nc.vector.tensor_scalar(
                out=lnse,
                in0=sumexp.bitcast(i32),
                scalar1=FLN_K,
                scalar2=FLN_C,
                op0=ALU.mult,
                op1=ALU.subtract,
            )
            nll = const.tile([P, RT], f32)
            nc.vector.tensor_sub(nll, lnse, picked)
            masked = const.tile([P, RT], f32)
            nc.vector.tensor_mul(masked, nll, mask_sb)

            stacked = const.tile([P, 2], f32)
            nc.vector.tensor_reduce(
                out=stacked[:, 0:1], in_=masked, axis=AX.X, op=ALU.add
            )
            nc.vector.tensor_reduce(
                out=stacked[:, 1:2], in_=mask_sb, axis=AX.X, op=ALU.add
            )

            ones = const.tile([P, 1], f32)
            nc.vector.memset(ones, 1.0)
            fin = psum.tile([P, 2048], f32, tag="ps", name="fin")
            nc.tensor.matmul(fin[0:1, 0:2], ones, stacked, start=True, stop=True)
            out_sb = const.tile([1, 2], f32)
            nc.vector.tensor_copy(out=out_sb, in_=fin[0:1, 0:2])
            nc.sync.dma_start(out=d_out, in_=out_sb)

    nc.compile()
    return nc


def get_nc(debug=False):
    key = ("nc", debug)
    if key not in _CACHE:
        _CACHE[key] = _build(debug=debug)
    return _CACHE[key]


def make_in_maps(inputs, label, ious, lut, cq, reliability):
    """Host-side shard prep. Index gathers / transposes / casts only."""
    inputs = np.asarray(inputs, dtype=np.float32)
    label = np.asarray(label).astype(np.int64)
    lut = np.asarray(lut, dtype=np.float32)
    cq = np.asarray(cq, dtype=np.float32)
    reliability = np.asarray(reliability, dtype=np.float32)

    bank = np.concatenate([lut, cq], axis=0)                 # [L, D]
    scaled = bank * (OIM_SCALAR * reliability)[:, None]      # [L, D] fp32
    # [P, KC, L] fp8: (p, k, j) = scaled[j, k*128+p]
    bank8 = np.ascontiguousarray(
        scaled.T.reshape(KC, P, L).transpose(1, 0, 2)
    ).astype(FP8)

    valid = label != IGNORE
    safe = np.where(valid, label, 0)
    bsel_full = scaled[safe].astype(BF16)                    # [N, D]
    inp_bf = inputs.astype(BF16)                             # [N, D]
    inp8_full = (inputs * FP8_SCALE).astype(FP8)             # [N, D]

    in_maps = []
    for c in range(NCORES):
        sl = slice(c * NSH, (c + 1) * NSH)
        # [P, RT, KC, P]: (p, rt, k, c) = x16[rt*128+c, k*128+p]
        x8 = inp8_full[sl].astype(FP8)
        inp = np.ascontiguousarray(
            x8.T.reshape(KC, P, RT, P).transpose(1, 2, 0, 3)
        )
        x = inp_bf[sl]
        rows = np.ascontiguousarray(x.reshape(RT, P, D).transpose(1, 0, 2))
        bsel = np.ascontiguousarray(
            bsel_full[sl].reshape(RT, P, D).transpose(1, 0, 2)
        )
        mask = np.ascontiguousarray(
            valid[sl].reshape(RT, P).T.astype(np.float32)
        )
        in_maps.append(
            {"inp": inp, "bank": bank8, "rows": rows, "bsel": bsel, "mask": mask}
        )
    return in_maps


def _combine(parts):
    """parts: list of [1,2] arrays per core -> scalar loss."""
    arr = np.stack([np.asarray(p, dtype=np.float64) for p in parts])  # [8,1,2]
    total = arr[:, 0, 0].sum()
    count = arr[:, 0, 1].sum()
    return np.float32(total / max(count, 1.0))


def kernel(inputs, label, ious, lut, cq, reliability):
    from concourse import bass_utils

    nc = get_nc()
    in_maps = make_in_maps(inputs, label, ious, lut, cq, reliability)
    res = bass_utils.run_bass_kernel_spmd(nc, in_maps, core_ids=list(range(NCORES)))
    return _combine([r["out"] for r in res.results])
